# revision 1
# baseline (speedup 1.0000x reference)
"""TRN2 Bass kernel for GPT-style causal self-attention with RoPE.

Reference (B=2, S=2048, D=1024, H=16, dk=64):
  qkv = hidden @ c_attn_w + c_attn_b; rope(q), rope(k) via position_ids;
  out = softmax(causal(q k^T / 8)) v, merged heads, @ c_proj_w + c_proj_b.

Sharding across 8 NeuronCores: core c = 4*b + g handles batch b and head
group g (4 heads = 256 dims). Each core computes its full S x S attention
and a row-sliced c_proj partial; the host sums the 4 partials per batch.

Device pipeline per core (all matmuls float32r):
  1. QKV in natural layout from host-transposed hiddenT (lhsT = hiddenT
     chunks); bias via K=1 ones-row matmul; rope applied in natural layout
     (2 DVE multiplies using a pair-swap access pattern); the rope add is
     folded into two accumulated PE transposes -> qT/kT [2-head dk, S];
     v evicted to [s, 4 x 65] layout with a ones column per head.
  2. Per head-pair, per 512-wide q chunk: scores^T via row-tiled K=64
     matmul pairs (two heads concurrently in the PE array); exp on ScalarE
     (scale=1/8); causal diagonal mask (0/1) on GPSIMD post-exp;
     PV: out[0:65] = [v | ones]^T @ P^T accumulated over k blocks (row 64 =
     softmax denominators); normalize via reciprocal_approx_fast + GPSIMD
     partition_broadcast + DVE multiply.
  3. Transposed projection projT[d, s] = Wp_chunk^T @ attn^T with c_proj_b
     added through the Identity-activation per-partition bias.
Output per core: outT [1024, 2048] partial; host sums per batch, transposes.
"""

from contextlib import ExitStack

import numpy as np

import concourse.bacc as bacc
import concourse.tile as tile
import concourse.mybir as mybir
from concourse.bass_utils import run_bass_kernel_spmd

f32 = mybir.dt.float32
f32r = mybir.dt.float32r
AF = mybir.ActivationFunctionType
ALU = mybir.AluOpType

S = 2048
D = 1024
HD = 256           # head dims per core (4 heads x 64)
SB = S // 128      # 16
KC = D // 128      # 8
NCH = S // 512     # 4


def build_attention_nc(num_devices=8):
    nc = bacc.Bacc("TRN2", target_bir_lowering=False, debug=False,
                   num_devices=num_devices)

    hT_d = nc.dram_tensor("hT", [D, S], f32r, kind="ExternalInput")
    wqkv_d = nc.dram_tensor("wqkv", [D, 768], f32r, kind="ExternalInput")
    bqkv_d = nc.dram_tensor("bqkv", [1, 768], f32r, kind="ExternalInput")
    cos4_d = nc.dram_tensor("cos4", [S, HD], f32r, kind="ExternalInput")
    sins4_d = nc.dram_tensor("sins4", [S, HD], f32r, kind="ExternalInput")
    wp_d = nc.dram_tensor("wp", [HD, D], f32r, kind="ExternalInput")
    bp_d = nc.dram_tensor("bp", [128, 8], f32, kind="ExternalInput")
    mask01_d = nc.dram_tensor("mask01", [128, 128], f32r, kind="ExternalInput")
    ones64_d = nc.dram_tensor("ones64", [128, 64], f32r, kind="ExternalInput")
    ident_d = nc.dram_tensor("ident", [128, 128], f32r, kind="ExternalInput")
    onesrow_d = nc.dram_tensor("ones_row", [1, 128], f32r, kind="ExternalInput")
    outT_d = nc.dram_tensor("outT", [D, S], f32, kind="ExternalOutput")

    with tile.TileContext(nc) as tc, ExitStack() as top:
        const = top.enter_context(tc.tile_pool(name="const", bufs=1))
        ident = const.tile([128, 128], f32r, tag="ident")
        nc.sync.dma_start(ident[:], ident_d.ap())
        mask01 = const.tile([128, 128], f32r, tag="mask01")
        nc.sync.dma_start(mask01[:], mask01_d.ap())
        ones_row = const.tile([1, 128], f32r, tag="ones_row")
        nc.sync.dma_start(ones_row[:], onesrow_d.ap())
        bp_sb = const.tile([128, 8], f32, tag="bp")
        nc.sync.dma_start(bp_sb[:], bp_d.ap())

        persist = top.enter_context(tc.tile_pool(name="persist", bufs=1))
        qT = [persist.tile([128, S], f32r, tag=f"qT{hp}", name=f"qT{hp}")
              for hp in range(2)]
        kT = [persist.tile([128, S], f32r, tag=f"kT{hp}", name=f"kT{hp}")
              for hp in range(2)]
        v_sb = persist.tile([128, SB, 4, 65], f32r, tag="v")
        ones64 = const.tile([128, 64], f32r, tag="ones64")
        nc.sync.dma_start(ones64[:], ones64_d.ap())
        nc.scalar.copy(v_sb[:, :, :, 64],
                       ones64[:].rearrange("p (a b) -> p a b", a=SB))
        wp_sb = persist.tile([128, 2, D], f32r, tag="wp")
        for kc2 in range(2):
            nc.sync.dma_start(wp_sb[:, kc2, :],
                              wp_d.ap()[kc2 * 128:(kc2 + 1) * 128, :])

        # ============ stage 1: QKV + rope + transpose ============
        with ExitStack() as st1:
            hT_pool = st1.enter_context(tc.tile_pool(name="hT", bufs=1))
            w_pool = st1.enter_context(tc.tile_pool(name="w", bufs=1))
            trig_pool = st1.enter_context(tc.tile_pool(name="trig", bufs=2))
            qkv_ps = st1.enter_context(
                tc.tile_pool(name="qkv_ps", bufs=2, space="PSUM"))
            tr_ps = st1.enter_context(
                tc.tile_pool(name="tr_ps", bufs=2, space="PSUM"))
            rope_pool = st1.enter_context(tc.tile_pool(name="rope", bufs=1))

            hT_sb = [hT_pool.tile([128, S], f32r, tag=f"hT{kc}", name=f"hT{kc}")
                     for kc in range(KC)]
            for kc in range(KC):
                nc.sync.dma_start(hT_sb[kc][:],
                                  hT_d.ap()[kc * 128:(kc + 1) * 128, :])
            w_sb = [w_pool.tile([128, 768], f32r, tag=f"w{kc}", name=f"w{kc}")
                    for kc in range(KC)]
            for kc in range(KC):
                nc.sync.dma_start(w_sb[kc][:],
                                  wqkv_d.ap()[kc * 128:(kc + 1) * 128, :])
            bqkv_sb = w_pool.tile([1, 768], f32r, tag="bqkv")
            nc.sync.dma_start(bqkv_sb[:], bqkv_d.ap())

            for sg in range(SB // 4):
                rope_tiles = {}
                for sbl in range(4):
                    sb = sg * 4 + sbl
                    cos_t = trig_pool.tile([128, HD], f32r, tag=f"cos{sbl}",
                                           name=f"cos{sbl}")
                    sins_t = trig_pool.tile([128, HD], f32r, tag=f"sin{sbl}",
                                            name=f"sin{sbl}")
                    nc.sync.dma_start(
                        cos_t[:], cos4_d.ap()[sb * 128:(sb + 1) * 128, :])
                    nc.sync.dma_start(
                        sins_t[:], sins4_d.ap()[sb * 128:(sb + 1) * 128, :])

                    qkv_p = qkv_ps.tile([128, 768], f32, tag="qkv_p")
                    for kc in range(KC):
                        lhsT = hT_sb[kc][:, sb * 128:(sb + 1) * 128]
                        nc.tensor.matmul(qkv_p[:, 0:512], lhsT,
                                         w_sb[kc][:, 0:512],
                                         start=(kc == 0), stop=False)
                        nc.tensor.matmul(qkv_p[:, 512:768], lhsT,
                                         w_sb[kc][:, 512:768],
                                         start=(kc == 0), stop=False)
                    nc.tensor.matmul(qkv_p[:, 0:512], ones_row[:],
                                     bqkv_sb[:, 0:512], start=False, stop=True)
                    nc.tensor.matmul(qkv_p[:, 512:768], ones_row[:],
                                     bqkv_sb[:, 512:768], start=False,
                                     stop=True)

                    for qk in range(2):
                        base = qk * HD
                        pin = qkv_p[:, base:base + HD]
                        pin_sw = qkv_p[:, base:base + HD].rearrange(
                            "p (h t d) -> p h t d", h=4, t=2)[:, :, ::-1, :]
                        t1 = rope_pool.tile([128, HD], f32r,
                                            tag=f"t1_{qk}_{sbl}",
                                            name=f"t1_{qk}_{sbl}")
                        t2 = rope_pool.tile([128, HD], f32r,
                                            tag=f"t2_{qk}_{sbl}",
                                            name=f"t2_{qk}_{sbl}")
                        nc.vector.tensor_tensor(t1[:], pin, cos_t[:],
                                                op=ALU.mult)
                        nc.vector.tensor_tensor(
                            t2[:].rearrange("p (h t d) -> p h t d", h=4, t=2),
                            pin_sw,
                            sins_t[:].rearrange("p (h t d) -> p h t d",
                                                h=4, t=2),
                            op=ALU.mult)
                        rope_tiles[(qk, sbl)] = (t1, t2)

                    nc.scalar.copy(
                        v_sb[:, sb, :, 0:64],
                        qkv_p[:, 512:768].rearrange("p (h d) -> p h d", h=4))

                for qk in range(2):
                    dest = qT if qk == 0 else kT
                    for hp in range(2):
                        tp = tr_ps.tile([128, 512], f32, tag="tr_p")
                        for sbl in range(4):
                            t1, t2 = rope_tiles[(qk, sbl)]
                            dst = tp[:, sbl * 128:(sbl + 1) * 128].bitcast(f32r)
                            nc.tensor.matmul(
                                dst, t1[:, hp * 128:(hp + 1) * 128], ident[:],
                                is_transpose=True, start=True, stop=False)
                            nc.tensor.matmul(
                                dst, t2[:, hp * 128:(hp + 1) * 128], ident[:],
                                is_transpose=True, start=False, stop=True)
                        nc.any.tensor_copy(
                            dest[hp][:, sg * 512:(sg + 1) * 512], tp[:])

        # ============ stages 2+3 ============
        st23 = top.enter_context(ExitStack())
        a_pool = st23.enter_context(tc.tile_pool(name="a_pool", bufs=1))
        aT2 = [a_pool.tile([128, S], f32r, tag=f"aT2{hp}", name=f"aT2{hp}")
               for hp in range(2)]
        aTo = [a_pool.tile([64, S], f32r, tag=f"aTo{hp}", name=f"aTo{hp}")
               for hp in range(2)]
        # ============ stage 2: attention ============
        with ExitStack() as st2:
            pt_pool = st2.enter_context(tc.tile_pool(name="pt", bufs=17))
            st_ps = st2.enter_context(
                tc.tile_pool(name="st_ps", bufs=2, space="PSUM"))
            out_ps = st2.enter_context(
                tc.tile_pool(name="out_ps", bufs=2, space="PSUM"))
            nrm_pool = st2.enter_context(tc.tile_pool(name="nrm", bufs=3))

            for hp in range(2):
                for c in range(NCH):
                    nkb = 4 * c + 4
                    pts = []
                    for kb in range(nkb):
                        q0 = max(512 * c, 128 * kb)
                        off = q0 - 512 * c
                        st_p = st_ps.tile([128, 2, 512], f32, tag="st_p")
                        for h2 in range(2):
                            nc.tensor.matmul(
                                st_p[:, h2, off:512],
                                kT[hp][h2 * 64:(h2 + 1) * 64,
                                       kb * 128:(kb + 1) * 128],
                                qT[hp][h2 * 64:(h2 + 1) * 64,
                                       q0:512 * (c + 1)],
                                start=True, stop=True,
                                tile_position=(h2 * 64, 0))
                        pt = pt_pool.tile([128, 2, 512], f32r, tag="pt")
                        nc.scalar.activation(pt[:, :, off:512],
                                             st_p[:, :, off:512],
                                             AF.Exp, scale=0.125)
                        if 128 * kb >= 512 * c:
                            for h2 in range(2):
                                nc.gpsimd.tensor_mul(
                                    pt[:, h2, off:off + 128],
                                    pt[:, h2, off:off + 128],
                                    mask01[:])
                        pts.append((kb, off, pt))

                    for h2 in range(2):
                        h = 2 * hp + h2
                        o_p = out_ps.tile([128, 512], f32, tag="o_p")
                        for (kb, off, pt) in pts:
                            nc.tensor.matmul(
                                o_p[0:65, off:512],
                                v_sb[:, kb, h, :],
                                pt[:, h2, off:512],
                                start=(kb == 0), stop=(kb == nkb - 1))
                        den = nrm_pool.tile([65, 512], f32, tag="den")
                        den0 = nrm_pool.tile([1, 512], f32, tag="den0")
                        rcp0 = nrm_pool.tile([1, 512], f32, tag="rcp0")
                        bc = nrm_pool.tile([64, 512], f32, tag="bc")
                        nc.scalar.copy(den[64:65, :], o_p[64:65, :])
                        # custom-DVE recip and partition_broadcast need
                        # partition-0 operands; DMA does the cross-partition hop
                        nc.sync.dma_start(den0[:], den[64:65, :])
                        nc.vector.reciprocal_approx_fast(rcp0[:], den0[:])
                        nc.gpsimd.partition_broadcast(bc[:], rcp0[:])
                        if h2 == 0:
                            out_ap = aT2[hp][0:64, c * 512:(c + 1) * 512]
                        else:
                            out_ap = aTo[hp][0:64, c * 512:(c + 1) * 512]
                        nc.vector.tensor_tensor(out_ap, o_p[0:64, :], bc[:],
                                                op=ALU.mult)

            for hp in range(2):
                nc.sync.dma_start(aT2[hp][64:128, :], aTo[hp][:])

        # ============ stage 3: projection ============
        with ExitStack() as st3:
            pj_ps = st3.enter_context(
                tc.tile_pool(name="pj_ps", bufs=2, space="PSUM"))
            pj_sb = st3.enter_context(tc.tile_pool(name="pj_sb", bufs=3))
            for dd in range(8):
                for sc in range(NCH):
                    pp = pj_ps.tile([128, 512], f32, tag="pp")
                    for kc2 in range(2):
                        nc.tensor.matmul(
                            pp[:],
                            wp_sb[:, kc2, dd * 128:(dd + 1) * 128],
                            aT2[kc2][:, sc * 512:(sc + 1) * 512],
                            start=(kc2 == 0), stop=(kc2 == 1))
                    po = pj_sb.tile([128, 512], f32, tag="po")
                    nc.scalar.activation(po[:], pp[:], AF.Identity,
                                         bias=bp_sb[:, dd:dd + 1])
                    nc.sync.dma_start(
                        outT_d.ap()[dd * 128:(dd + 1) * 128,
                                    sc * 512:(sc + 1) * 512],
                        po[:])

    nc.finalize()
    return nc


def make_core_inputs(inputs, core):
    """Host-side shard prep for one core."""
    b, g = core // 4, core % 4
    hidden = np.asarray(inputs["hidden_states"], dtype=np.float32)
    pos = np.asarray(inputs["position_ids"])
    caw = np.asarray(inputs["c_attn_w"], dtype=np.float32)
    cab = np.asarray(inputs["c_attn_b"], dtype=np.float32)
    cpw = np.asarray(inputs["c_proj_w"], dtype=np.float32)
    cpb = np.asarray(inputs["c_proj_b"], dtype=np.float32)

    cs = slice(g * HD, (g + 1) * HD)
    wqkv = np.concatenate(
        [caw[:, cs], caw[:, D + g * HD:D + (g + 1) * HD],
         caw[:, 2 * D + g * HD:2 * D + (g + 1) * HD]], axis=1)
    bqkv = np.concatenate(
        [cab[cs], cab[D + g * HD:D + (g + 1) * HD],
         cab[2 * D + g * HD:2 * D + (g + 1) * HD]])[None, :]

    inv_freq = (1.0 / (10000.0 **
                       (np.arange(0, 64, 2, dtype=np.float64) / 64.0)))
    freqs = pos[b].astype(np.float64)[:, None] * inv_freq[None, :]
    emb = np.concatenate([freqs, freqs], axis=1)
    cos = np.cos(emb).astype(np.float32)
    sin = np.sin(emb).astype(np.float32)
    sins = sin.copy()
    sins[:, :32] *= -1.0
    cos4 = np.tile(cos, (1, 4)).astype(np.float32)
    sins4 = np.tile(sins, (1, 4)).astype(np.float32)

    bp = (cpb if g == 0 else np.zeros_like(cpb)).reshape(8, 128).T.copy()

    r = np.arange(128)
    mask01 = (r[None, :] >= r[:, None]).astype(np.float32)

    return {
        "hT": np.ascontiguousarray(hidden[b].T),
        "wqkv": np.ascontiguousarray(wqkv),
        "bqkv": np.ascontiguousarray(bqkv),
        "cos4": cos4,
        "sins4": sins4,
        "wp": np.ascontiguousarray(cpw[cs, :]),
        "bp": np.ascontiguousarray(bp.astype(np.float32)),
        "mask01": mask01,
        "ones64": np.ones((128, 64), np.float32),
        "ident": np.eye(128, dtype=np.float32),
        "ones_row": np.ones((1, 128), np.float32),
    }


_NC_CACHE = {}


def run(inputs, trace=False, **spmd_kwargs):
    """Shard, execute on 8 cores, unshard. Returns (output, BassKernelResults)."""
    if "nc" not in _NC_CACHE:
        _NC_CACHE["nc"] = build_attention_nc(num_devices=8)
    nc = _NC_CACHE["nc"]
    in_maps = [make_core_inputs(inputs, c) for c in range(8)]
    res = run_bass_kernel_spmd(nc, in_maps, core_ids=list(range(8)),
                               trace=trace, **spmd_kwargs)
    outs = []
    for b in range(2):
        acc = np.zeros((D, S), np.float64)
        for g in range(4):
            acc += res.results[b * 4 + g]["outT"].astype(np.float64)
        outs.append(acc.T.astype(np.float32))
    return np.stack(outs, axis=0), res


def kernel(**inputs) -> np.ndarray:
    out, _ = run(inputs, trace=False)
    return out



# revision 16
# speedup vs baseline: 1.8627x; 1.8627x over previous
"""TRN2 Bass kernel for GPT-style causal self-attention with RoPE (bf16).

Reference (B=2, S=2048, D=1024, H=16, dk=64):
  qkv = hidden @ c_attn_w + c_attn_b; rope(q), rope(k) via position_ids;
  out = softmax(causal(q k^T / 8)) v, merged heads, @ c_proj_w + c_proj_b.

Sharding across 8 NeuronCores: core c = 4*b + g handles batch b and head
group g (4 heads = 256 dims). Each core computes its full S x S attention
and a row-sliced c_proj partial; the host sums the 4 partials per batch.

Kernel structure (all matmul operands bf16, fp32 PSUM accumulation):
  1. q^T/k^T computed directly in transposed layout (w chunks stationary,
     hT moving), with head dims pair-interleaved (rows 2i/2i+1 = dims
     i/i+32) so the rope partner sits on the adjacent partition; rope =
     stream_shuffle + 2 muls + add on DVE against host-prepped cosT/sinT.
     v computed in natural [s, d] layout (hT stationary), ones column
     appended for softmax denominators.
  2. Per head-pair, per 512-wide q chunk: scores^T via row-tiled K=64
     matmul pairs (two heads concurrent in the PE); exp (scale 1/8) on
     ScalarE -> bf16 P; causal diagonal 0/1 mask on GPSIMD post-exp;
     PV: out[0:65] = [v | ones]^T @ P^T accumulated over k blocks (row 64
     = denominators); normalize via reciprocal_approx_fast from PSUM +
     K=1 PE broadcast matmul + DVE multiply.
  3. projT[d, s] = Wp_chunk^T @ attn^T + bias (ACT), bf16 out DMA.
Stages are emitted interleaved (sg0, sg1, c0, sg2, c1, p0, ...) so the PE
stays dense (HAM stays warm); warmup matmuls cover the initial hT DMA.
"""

from contextlib import ExitStack

import numpy as np

import concourse.bacc as bacc
import concourse.tile as tile
import concourse.mybir as mybir
from concourse.bass_utils import run_bass_kernel_spmd

f32 = mybir.dt.float32
f32r = mybir.dt.float32r
bf16 = mybir.dt.bfloat16
AF = mybir.ActivationFunctionType
ALU = mybir.AluOpType

S = 2048
D = 1024
HD = 256           # head dims per core (4 heads x 64)
SB = S // 128      # 16
KC = D // 128      # 8
NCH = S // 512     # 4
SWAP_MASK = [i ^ 1 for i in range(32)]  # pair swap within quadrant


def build_attention_nc(num_devices=8):
    nc = bacc.Bacc("TRN2", target_bir_lowering=False, debug=False,
                   num_devices=num_devices)

    hT_d = nc.dram_tensor("hT", [D, S], bf16, kind="ExternalInput")
    wqkv_d = nc.dram_tensor("wqkv", [D, 768], bf16, kind="ExternalInput")
    bqk_d = nc.dram_tensor("bqk", [128, 4], f32, kind="ExternalInput")
    bv_d = nc.dram_tensor("bv", [1, 256], bf16, kind="ExternalInput")
    cosT_d = nc.dram_tensor("cosT", [128, S], bf16, kind="ExternalInput")
    sinT_d = nc.dram_tensor("sinT", [128, S], bf16, kind="ExternalInput")
    wp_d = nc.dram_tensor("wp", [HD, D], bf16, kind="ExternalInput")
    bp_d = nc.dram_tensor("bp", [128, 8], f32, kind="ExternalInput")
    mask2_d = nc.dram_tensor("mask2", [128, 256], bf16, kind="ExternalInput")
    ones64_d = nc.dram_tensor("ones64", [128, 64], bf16, kind="ExternalInput")
    onesrow_d = nc.dram_tensor("ones_row", [1, 128], bf16, kind="ExternalInput")
    outT_d = nc.dram_tensor("outT", [D, S], bf16, kind="ExternalOutput")
    warm_d = nc.dram_tensor("warm", [128, 512], bf16, kind="ExternalOutput")

    with tile.TileContext(nc) as tc, ExitStack() as top:
        const = top.enter_context(tc.tile_pool(name="const", bufs=1))
        mask2 = const.tile([128, 256], bf16, tag="mask2")
        nc.sync.dma_start(mask2[:], mask2_d.ap())
        ones_row = const.tile([1, 128], bf16, tag="ones_row")
        nc.sync.dma_start(ones_row[:], onesrow_d.ap())
        ones64 = const.tile([128, 64], bf16, tag="ones64")
        nc.sync.dma_start(ones64[:], ones64_d.ap())
        bqk_sb = const.tile([128, 4], f32, tag="bqk")
        nc.sync.dma_start(bqk_sb[:], bqk_d.ap())
        bv_sb = const.tile([1, 256], bf16, tag="bv")
        nc.sync.dma_start(bv_sb[:], bv_d.ap())
        bp_sb = const.tile([128, 8], f32, tag="bp")
        nc.sync.dma_start(bp_sb[:], bp_d.ap())

        persist = top.enter_context(tc.tile_pool(name="persist", bufs=1))
        w_sb = [persist.tile([128, 768], bf16, tag=f"w{dc}", name=f"w{dc}")
                for dc in range(KC)]
        for dc in range(KC):
            nc.sync.dma_start(w_sb[dc][:],
                              wqkv_d.ap()[dc * 128:(dc + 1) * 128, :])
        hT_sb = [persist.tile([128, S], bf16, tag=f"hT{dc}", name=f"hT{dc}")
                 for dc in range(KC)]
        # s-sliced hT DMA: sg0's chunks land first so QKV starts early
        for sg in range(NCH):
            for dc in range(KC):
                nc.sync.dma_start(
                    hT_sb[dc][:, sg * 512:(sg + 1) * 512],
                    hT_d.ap()[dc * 128:(dc + 1) * 128,
                              sg * 512:(sg + 1) * 512])
            if sg == 0:
                cosT_sb = persist.tile([128, S], bf16, tag="cosT")
                nc.sync.dma_start(cosT_sb[:], cosT_d.ap())
                sinT_sb = persist.tile([128, S], bf16, tag="sinT")
                nc.sync.dma_start(sinT_sb[:], sinT_d.ap())
        wp_sb = persist.tile([128, 2, D], bf16, tag="wp")
        for kc2 in range(2):
            nc.sync.dma_start(wp_sb[:, kc2, :],
                              wp_d.ap()[kc2 * 128:(kc2 + 1) * 128, :])

        qT = [persist.tile([128, S], bf16, tag=f"qT{hp}", name=f"qT{hp}")
              for hp in range(2)]
        kT = [persist.tile([128, S], bf16, tag=f"kT{hp}", name=f"kT{hp}")
              for hp in range(2)]
        v_sb = persist.tile([128, SB, 4, 66], bf16, tag="v")
        nc.scalar.copy(v_sb[:, :, :, 64],
                       ones64[:].rearrange("p (a b) -> p a b", a=SB))
        aT = [persist.tile([128, S], bf16, tag=f"aT{hp}", name=f"aT{hp}")
              for hp in range(2)]
        aTo = [persist.tile([64, S], bf16, tag=f"aTo{hp}", name=f"aTo{hp}")
               for hp in range(2)]

        # PSUM pools: 4 + 4 = 8 banks
        mm512 = top.enter_context(tc.tile_pool(name="mm512", bufs=4,
                                               space="PSUM"))
        stps = top.enter_context(tc.tile_pool(name="stps", bufs=2,
                                              space="PSUM"))
        # SBUF working pools
        work = top.enter_context(tc.tile_pool(name="work", bufs=2))
        pt_pool = top.enter_context(tc.tile_pool(name="pt", bufs=24))
        nrm = top.enter_context(tc.tile_pool(name="nrm", bufs=3))
        po_pool = top.enter_context(tc.tile_pool(name="po", bufs=3))

        # ---- warmup: keep PE busy during initial DMA; preload exp table ----
        junk = const.tile([128, 512], bf16, tag="junk")
        nc.vector.memset(junk[:], 0.0)
        junk_e = const.tile([1, 16], bf16, tag="junk_e")
        nc.scalar.activation(junk_e[:], junk[0:1, 0:16], AF.Exp, scale=0.125)
        warm_ps = mm512.tile([128, 512], f32, tag="mm512")
        for i in range(16):
            nc.tensor.matmul(warm_ps[:], junk[:, 0:128], junk[:],
                             start=(i == 0), stop=(i == 15))
        warm_sb = const.tile([128, 512], bf16, tag="warm_sb")
        nc.vector.tensor_copy(warm_sb[:], warm_ps[:])
        nc.sync.dma_start(warm_d.ap(), warm_sb[:])

        def stage1(sg):
            ssl = slice(sg * 512, (sg + 1) * 512)
            for jc in range(4):
                acc = mm512.tile([128, 512], f32, tag="mm512")
                for dc in range(KC):
                    nc.tensor.matmul(acc[:],
                                     w_sb[dc][:, jc * 128:(jc + 1) * 128],
                                     hT_sb[dc][:, ssl],
                                     start=(dc == 0), stop=(dc == KC - 1))
                raw = work.tile([128, 512], bf16, tag="raw")
                nc.scalar.activation(raw[:], acc[:], AF.Identity,
                                     bias=bqk_sb[:, jc:jc + 1])
                shuf = work.tile([128, 512], bf16, tag="shuf")
                nc.vector.stream_shuffle(shuf[:], raw[:], mask=SWAP_MASK)
                m1 = work.tile([128, 512], bf16, tag="m1")
                nc.vector.tensor_tensor(m1[:], raw[:], cosT_sb[:, ssl],
                                        op=ALU.mult)
                dest = (qT if jc < 2 else kT)[jc % 2]
                m2 = work.tile([128, 512], bf16, tag="m2")
                nc.vector.tensor_tensor(m2[:], shuf[:], sinT_sb[:, ssl],
                                        op=ALU.mult)
                nc.vector.tensor_tensor(dest[:, ssl], m1[:], m2[:],
                                        op=ALU.add)
            for sbl in range(4):
                sb = sg * 4 + sbl
                vp = mm512.tile([128, 256], f32, tag="mm512")
                for dc in range(KC):
                    nc.tensor.matmul(vp[:],
                                     hT_sb[dc][:, sb * 128:(sb + 1) * 128],
                                     w_sb[dc][:, 512:768],
                                     start=(dc == 0), stop=False)
                nc.tensor.matmul(vp[:], ones_row[:], bv_sb[:],
                                 start=False, stop=True)
                nc.vector.tensor_copy(
                    v_sb[:, sb, :, 0:64],
                    vp[:].rearrange("p (h d) -> p h d", h=4))

        def stage2(c):
            csl = slice(c * 512, (c + 1) * 512)
            nkb = 4 * c + 4
            for hp in range(2):
                pts = []
                for kb in range(nkb):
                    q0 = max(512 * c, 128 * kb)
                    off = q0 - 512 * c
                    st_p = stps.tile([128, 2, 512], f32, tag="st")
                    for h2 in range(2):
                        nc.tensor.matmul(
                            st_p[:, h2, off:512],
                            kT[hp][h2 * 64:(h2 + 1) * 64,
                                   kb * 128:(kb + 1) * 128],
                            qT[hp][h2 * 64:(h2 + 1) * 64,
                                   q0:512 * (c + 1)],
                            start=True, stop=True,
                            tile_position=(h2 * 64, 0))
                    pt = pt_pool.tile([128, 2, 512], bf16, tag="pt")
                    nc.scalar.activation(pt[:, :, off:512],
                                         st_p[:, :, off:512],
                                         AF.Exp, scale=0.125)
                    if 128 * kb >= 512 * c:
                        nc.vector.tensor_tensor(
                            pt[:, :, off:off + 128],
                            pt[:, :, off:off + 128],
                            mask2[:].rearrange("p (a b) -> p a b", a=2),
                            op=ALU.mult)
                    pts.append((kb, off, pt))

                for h2 in range(2):
                    h = 2 * hp + h2
                    o_p = mm512.tile([128, 512], f32, tag="mm512")
                    for (kb, off, pt) in pts:
                        nc.tensor.matmul(
                            o_p[0:65, off:512],
                            v_sb[:, kb, h, 0:65],
                            pt[:, h2, off:512],
                            start=(kb == 0), stop=(kb == nkb - 1))
                    # den row 64 -> partition 0 (custom-DVE recip and
                    # partition_broadcast need partition-0 operands)
                    den64 = nrm.tile([65, 512], f32, tag="den64")
                    nc.vector.tensor_copy(den64[64:65, :], o_p[64:65, :])
                    den0 = nrm.tile([1, 512], f32, tag="den0")
                    nc.sync.dma_start(den0[:], den64[64:65, :])
                    rcp = nrm.tile([1, 512], f32, tag="rcp")
                    nc.vector.reciprocal_approx_fast(rcp[:], den0[:])
                    bc = nrm.tile([64, 512], f32, tag="bc")
                    nc.gpsimd.partition_broadcast(bc[:], rcp[:])
                    dest = (aT[hp][0:64, csl] if h2 == 0
                            else aTo[hp][:, csl])
                    nc.vector.tensor_tensor(dest, o_p[0:64, :], bc[:],
                                            op=ALU.mult)
            for hp in range(2):
                nc.gpsimd.dma_start(aT[hp][64:128, csl], aTo[hp][:, csl])

        def stage3(sc):
            scl = slice(sc * 512, (sc + 1) * 512)
            for dd in range(8):
                pp = mm512.tile([128, 512], f32, tag="mm512")
                for kc2 in range(2):
                    nc.tensor.matmul(
                        pp[:],
                        wp_sb[:, kc2, dd * 128:(dd + 1) * 128],
                        aT[kc2][:, scl],
                        start=(kc2 == 0), stop=(kc2 == 1))
                po = po_pool.tile([128, 512], bf16, tag="po")
                nc.scalar.activation(po[:], pp[:], AF.Identity,
                                     bias=bp_sb[:, dd:dd + 1])
                nc.sync.dma_start(
                    outT_d.ap()[dd * 128:(dd + 1) * 128, scl], po[:])

        # interleaved emission: keeps PE dense, lets exp start early;
        # proj chunks late so PE has fill work during the exp-bound tail
        stage1(0)
        stage1(1)
        stage2(0)
        stage1(2)
        stage2(1)
        stage1(3)
        stage2(2)
        stage3(0)
        stage3(1)
        stage2(3)
        stage3(2)
        stage3(3)

    nc.finalize()
    return nc


# pair-interleave: new row j within a head holds original dim PERM[j]
PERM = np.empty(64, np.int64)
PERM[0::2] = np.arange(32)
PERM[1::2] = np.arange(32) + 32


def make_core_inputs(inputs, core):
    """Host-side shard prep for one core."""
    import ml_dtypes
    bf = ml_dtypes.bfloat16
    b, g = core // 4, core % 4
    hidden = np.asarray(inputs["hidden_states"], dtype=np.float32)
    pos = np.asarray(inputs["position_ids"])
    caw = np.asarray(inputs["c_attn_w"], dtype=np.float32)
    cab = np.asarray(inputs["c_attn_b"], dtype=np.float32)
    cpw = np.asarray(inputs["c_proj_w"], dtype=np.float32)
    cpb = np.asarray(inputs["c_proj_b"], dtype=np.float32)

    cs = slice(g * HD, (g + 1) * HD)
    # per-head pair-interleaved column permutation for q and k
    hperm = np.concatenate([h * 64 + PERM for h in range(4)])
    wq = caw[:, cs][:, hperm]
    wk = caw[:, D + g * HD:D + (g + 1) * HD][:, hperm]
    wv = caw[:, 2 * D + g * HD:2 * D + (g + 1) * HD]
    wqkv = np.concatenate([wq, wk, wv], axis=1)

    bq = cab[cs][hperm]
    bk = cab[D + g * HD:D + (g + 1) * HD][hperm]
    bv = cab[2 * D + g * HD:2 * D + (g + 1) * HD]
    # bqk[:, jc]: jc0/1 = q head pairs, jc2/3 = k head pairs
    bqk = np.stack([bq[0:128], bq[128:256], bk[0:128], bk[128:256]],
                   axis=1).astype(np.float32)

    # rope tables in permuted transposed layout [128 rows = 2 heads x 64]
    inv_freq = (1.0 / (10000.0 **
                       (np.arange(0, 64, 2, dtype=np.float64) / 64.0)))
    theta = pos[b].astype(np.float64)[None, :] * inv_freq[:, None]  # [32,S]
    cosv = np.cos(theta)
    sinv = np.sin(theta)
    cos64 = np.empty((64, S), np.float64)
    sin64 = np.empty((64, S), np.float64)
    cos64[0::2] = cosv
    cos64[1::2] = cosv
    sin64[0::2] = -sinv      # row 2i   (orig dim i):    -sin
    sin64[1::2] = sinv       # row 2i+1 (orig dim i+32): +sin
    cosT = np.tile(cos64, (2, 1)).astype(bf)
    sinT = np.tile(sin64, (2, 1)).astype(bf)

    bp = (cpb if g == 0 else np.zeros_like(cpb)).reshape(8, 128).T.copy()

    r = np.arange(128)
    mask01 = (r[None, :] >= r[:, None]).astype(np.float32)
    mask2 = np.concatenate([mask01, mask01], axis=1)

    return {
        "hT": np.ascontiguousarray(hidden[b].T).astype(bf),
        "wqkv": np.ascontiguousarray(wqkv).astype(bf),
        "bqk": np.ascontiguousarray(bqk),
        "bv": bv[None, :].astype(bf),
        "cosT": cosT,
        "sinT": sinT,
        "wp": np.ascontiguousarray(cpw[cs, :]).astype(bf),
        "bp": np.ascontiguousarray(bp.astype(np.float32)),
        "mask2": mask2.astype(bf),
        "ones64": np.ones((128, 64), bf),
        "ones_row": np.ones((1, 128), bf),
    }


_NC_CACHE = {}


def run(inputs, trace=False, **spmd_kwargs):
    """Shard, execute on 8 cores, unshard. Returns (output, BassKernelResults)."""
    if "nc" not in _NC_CACHE:
        _NC_CACHE["nc"] = build_attention_nc(num_devices=8)
    nc = _NC_CACHE["nc"]
    in_maps = [make_core_inputs(inputs, c) for c in range(8)]
    res = run_bass_kernel_spmd(nc, in_maps, core_ids=list(range(8)),
                               trace=trace, **spmd_kwargs)
    outs = []
    for b in range(2):
        acc = np.zeros((D, S), np.float64)
        for g in range(4):
            acc += res.results[b * 4 + g]["outT"].astype(np.float64)
        outs.append(acc.T.astype(np.float32))
    return np.stack(outs, axis=0), res


def kernel(**inputs) -> np.ndarray:
    out, _ = run(inputs, trace=False)
    return out


# revision 21
# speedup vs baseline: 1.9225x; 1.0321x over previous
"""TRN2 Bass kernel for GPT-style causal self-attention with RoPE (bf16).

Reference (B=2, S=2048, D=1024, H=16, dk=64):
  qkv = hidden @ c_attn_w + c_attn_b; rope(q), rope(k) via position_ids;
  out = softmax(causal(q k^T / 8)) v, merged heads, @ c_proj_w + c_proj_b.

Sharding across 8 NeuronCores: core c = 4*b + g handles batch b and head
group g (4 heads = 256 dims). Each core computes its full S x S attention
and a row-sliced c_proj partial; the host sums the 4 partials per batch.

Kernel structure (all matmul operands bf16, fp32 PSUM accumulation):
  1. q^T/k^T computed directly in transposed layout (w chunks stationary,
     hT moving), with head dims pair-interleaved (rows 2i/2i+1 = dims
     i/i+32) so the rope partner sits on the adjacent partition; rope =
     stream_shuffle + 2 muls + add on DVE against host-prepped cosT/sinT.
     v computed in natural [s, d] layout (hT stationary), ones column
     appended for softmax denominators.
  2. Per head-pair, per 512-wide q chunk: scores^T via row-tiled K=64
     matmul pairs (two heads concurrent in the PE); exp (scale 1/8) on
     ScalarE -> bf16 P; causal diagonal 0/1 mask on GPSIMD post-exp;
     PV: out[0:65] = [v | ones]^T @ P^T accumulated over k blocks (row 64
     = denominators); normalize via reciprocal_approx_fast from PSUM +
     K=1 PE broadcast matmul + DVE multiply.
  3. projT[d, s] = Wp_chunk^T @ attn^T + bias (ACT), bf16 out DMA.
Stages are emitted interleaved (sg0, sg1, c0, sg2, c1, p0, ...) so the PE
stays dense (HAM stays warm); warmup matmuls cover the initial hT DMA.
"""

from contextlib import ExitStack

import numpy as np

import concourse.bacc as bacc
import concourse.tile as tile
import concourse.mybir as mybir
from concourse.bass_utils import run_bass_kernel_spmd

f32 = mybir.dt.float32
f32r = mybir.dt.float32r
bf16 = mybir.dt.bfloat16
AF = mybir.ActivationFunctionType
ALU = mybir.AluOpType

S = 2048
D = 1024
HD = 256           # head dims per core (4 heads x 64)
SB = S // 128      # 16
KC = D // 128      # 8
NCH = S // 512     # 4
SWAP_MASK = [i ^ 1 for i in range(32)]  # pair swap within quadrant


def build_attention_nc(num_devices=8):
    nc = bacc.Bacc("TRN2", target_bir_lowering=False, debug=False,
                   num_devices=num_devices)

    hT_d = nc.dram_tensor("hT", [D, S], bf16, kind="ExternalInput")
    wqkv_d = nc.dram_tensor("wqkv", [D, 768], bf16, kind="ExternalInput")
    bqk_d = nc.dram_tensor("bqk", [128, 4], f32, kind="ExternalInput")
    bv_d = nc.dram_tensor("bv", [1, 256], bf16, kind="ExternalInput")
    cosT_d = nc.dram_tensor("cosT", [128, S], bf16, kind="ExternalInput")
    sinT_d = nc.dram_tensor("sinT", [128, S], bf16, kind="ExternalInput")
    wp_d = nc.dram_tensor("wp", [HD, D], bf16, kind="ExternalInput")
    bp_d = nc.dram_tensor("bp", [128, 8], f32, kind="ExternalInput")
    mask2_d = nc.dram_tensor("mask2", [128, 256], bf16, kind="ExternalInput")
    ones64_d = nc.dram_tensor("ones64", [128, 64], bf16, kind="ExternalInput")
    onesrow_d = nc.dram_tensor("ones_row", [1, 128], bf16, kind="ExternalInput")
    outT_d = nc.dram_tensor("outT", [D, S], bf16, kind="ExternalOutput")
    warm_d = nc.dram_tensor("warm", [128, 512], bf16, kind="ExternalOutput")

    with tile.TileContext(nc) as tc, ExitStack() as top:
        const = top.enter_context(tc.tile_pool(name="const", bufs=1))
        persist = top.enter_context(tc.tile_pool(name="persist", bufs=1))

        # batched DMAs, critical-path first: w, hT(sg0), consts, rest
        w_sb = persist.tile([128, KC, 768], bf16, tag="w")
        nc.sync.dma_start(w_sb[:],
                          wqkv_d.ap().rearrange("(a p) j -> p a j", a=KC))
        hT_sb = persist.tile([128, KC, S], bf16, tag="hT")
        hT_src = hT_d.ap().rearrange("(a p) s -> p a s", a=KC)
        nc.sync.dma_start(hT_sb[:, :, 0:512], hT_src[:, :, 0:512])

        mask2 = const.tile([128, 256], bf16, tag="mask2")
        nc.sync.dma_start(mask2[:], mask2_d.ap())
        ones_row = const.tile([1, 128], bf16, tag="ones_row")
        nc.sync.dma_start(ones_row[:], onesrow_d.ap())
        ones64 = const.tile([128, 64], bf16, tag="ones64")
        nc.sync.dma_start(ones64[:], ones64_d.ap())
        bqk_sb = const.tile([128, 4], f32, tag="bqk")
        nc.sync.dma_start(bqk_sb[:], bqk_d.ap())
        bv_sb = const.tile([1, 256], bf16, tag="bv")
        nc.sync.dma_start(bv_sb[:], bv_d.ap())
        bp_sb = const.tile([128, 8], f32, tag="bp")
        nc.sync.dma_start(bp_sb[:], bp_d.ap())

        cosT_sb = persist.tile([128, S], bf16, tag="cosT")
        nc.sync.dma_start(cosT_sb[:], cosT_d.ap())
        sinT_sb = persist.tile([128, S], bf16, tag="sinT")
        nc.sync.dma_start(sinT_sb[:], sinT_d.ap())
        for sg in range(1, NCH):
            ssl = slice(sg * 512, (sg + 1) * 512)
            nc.sync.dma_start(hT_sb[:, :, ssl], hT_src[:, :, ssl])
        wp_sb = persist.tile([128, 2, D], bf16, tag="wp")
        nc.sync.dma_start(wp_sb[:],
                          wp_d.ap().rearrange("(a p) j -> p a j", a=2))

        qT = [persist.tile([128, S], bf16, tag=f"qT{hp}", name=f"qT{hp}")
              for hp in range(2)]
        kT = [persist.tile([128, S], bf16, tag=f"kT{hp}", name=f"kT{hp}")
              for hp in range(2)]
        v_sb = persist.tile([128, SB, 4, 66], bf16, tag="v")
        nc.scalar.copy(v_sb[:, :, :, 64],
                       ones64[:].rearrange("p (a b) -> p a b", a=SB))
        aT = [persist.tile([128, S], bf16, tag=f"aT{hp}", name=f"aT{hp}")
              for hp in range(2)]
        aTo = [persist.tile([64, S], bf16, tag=f"aTo{hp}", name=f"aTo{hp}")
               for hp in range(2)]

        # PSUM pools: 4 + 4 = 8 banks
        mm512 = top.enter_context(tc.tile_pool(name="mm512", bufs=4,
                                               space="PSUM"))
        stps = top.enter_context(tc.tile_pool(name="stps", bufs=2,
                                              space="PSUM"))
        # SBUF working pools
        work = top.enter_context(tc.tile_pool(name="work", bufs=2))
        pt_pool = top.enter_context(tc.tile_pool(name="pt", bufs=20))
        nrm = top.enter_context(tc.tile_pool(name="nrm", bufs=2))
        po_pool = top.enter_context(tc.tile_pool(name="po", bufs=2))

        # ---- warmup: keep PE busy during initial DMA; preload exp table ----
        junk = const.tile([128, 512], bf16, tag="junk")
        nc.vector.memset(junk[:], 0.0)
        junk_e = const.tile([1, 16], bf16, tag="junk_e")
        nc.scalar.activation(junk_e[:], junk[0:1, 0:16], AF.Exp, scale=0.125)
        warm_ps = mm512.tile([128, 512], f32, tag="mm512")
        for i in range(16):
            nc.tensor.matmul(warm_ps[:], junk[:, 0:128], junk[:],
                             start=(i == 0), stop=(i == 15))
        warm_sb = const.tile([128, 512], bf16, tag="warm_sb")
        nc.vector.tensor_copy(warm_sb[:], warm_ps[:])
        nc.sync.dma_start(warm_d.ap(), warm_sb[:])

        def stage1(sg):
            ssl = slice(sg * 512, (sg + 1) * 512)
            for jc in range(4):
                acc = mm512.tile([128, 512], f32, tag="mm512")
                for dc in range(KC):
                    nc.tensor.matmul(acc[:],
                                     w_sb[:, dc, jc * 128:(jc + 1) * 128],
                                     hT_sb[:, dc, ssl],
                                     start=(dc == 0), stop=(dc == KC - 1))
                raw = work.tile([128, 512], bf16, tag="raw")
                nc.scalar.activation(raw[:], acc[:], AF.Identity,
                                     bias=bqk_sb[:, jc:jc + 1])
                shuf = work.tile([128, 512], bf16, tag="shuf")
                nc.vector.stream_shuffle(shuf[:], raw[:], mask=SWAP_MASK)
                m1 = work.tile([128, 512], bf16, tag="m1")
                nc.vector.tensor_tensor(m1[:], raw[:], cosT_sb[:, ssl],
                                        op=ALU.mult)
                dest = (qT if jc < 2 else kT)[jc % 2]
                m2 = work.tile([128, 512], bf16, tag="m2")
                nc.vector.tensor_tensor(m2[:], shuf[:], sinT_sb[:, ssl],
                                        op=ALU.mult)
                nc.vector.tensor_tensor(dest[:, ssl], m1[:], m2[:],
                                        op=ALU.add)
            for sbl in range(4):
                sb = sg * 4 + sbl
                vp = mm512.tile([128, 256], f32, tag="mm512")
                for dc in range(KC):
                    nc.tensor.matmul(vp[:],
                                     hT_sb[:, dc, sb * 128:(sb + 1) * 128],
                                     w_sb[:, dc, 512:768],
                                     start=(dc == 0), stop=False)
                nc.tensor.matmul(vp[:], ones_row[:], bv_sb[:],
                                 start=False, stop=True)
                nc.vector.tensor_copy(
                    v_sb[:, sb, :, 0:64],
                    vp[:].rearrange("p (h d) -> p h d", h=4))

        def stage2(c):
            csl = slice(c * 512, (c + 1) * 512)
            nkb = 4 * c + 4
            for hp in range(2):
                pts = []
                for kb in range(nkb):
                    q0 = max(512 * c, 128 * kb)
                    off = q0 - 512 * c
                    st_p = stps.tile([128, 2, 512], f32, tag="st")
                    for h2 in range(2):
                        nc.tensor.matmul(
                            st_p[:, h2, off:512],
                            kT[hp][h2 * 64:(h2 + 1) * 64,
                                   kb * 128:(kb + 1) * 128],
                            qT[hp][h2 * 64:(h2 + 1) * 64,
                                   q0:512 * (c + 1)],
                            start=True, stop=True,
                            tile_position=(h2 * 64, 0))
                    pt = pt_pool.tile([128, 2, 512], bf16, tag="pt")
                    nc.scalar.activation(pt[:, :, off:512],
                                         st_p[:, :, off:512],
                                         AF.Exp, scale=0.125)
                    if 128 * kb >= 512 * c:
                        nc.vector.tensor_tensor(
                            pt[:, :, off:off + 128],
                            pt[:, :, off:off + 128],
                            mask2[:].rearrange("p (a b) -> p a b", a=2),
                            op=ALU.mult)
                    pts.append((kb, off, pt))

                for h2 in range(2):
                    h = 2 * hp + h2
                    o_p = mm512.tile([128, 512], f32, tag="mm512")
                    for (kb, off, pt) in pts:
                        nc.tensor.matmul(
                            o_p[0:65, off:512],
                            v_sb[:, kb, h, 0:65],
                            pt[:, h2, off:512],
                            start=(kb == 0), stop=(kb == nkb - 1))
                    # den row 64 -> partition 0 (custom-DVE recip and
                    # partition_broadcast need partition-0 operands)
                    den64 = nrm.tile([65, 512], f32, tag="den64")
                    nc.vector.tensor_copy(den64[64:65, :], o_p[64:65, :])
                    den0 = nrm.tile([1, 512], f32, tag="den0")
                    nc.sync.dma_start(den0[:], den64[64:65, :])
                    rcp = nrm.tile([1, 512], f32, tag="rcp")
                    nc.vector.reciprocal_approx_fast(rcp[:], den0[:])
                    bc = nrm.tile([64, 512], f32, tag="bc")
                    nc.gpsimd.partition_broadcast(bc[:], rcp[:])
                    dest = (aT[hp][0:64, csl] if h2 == 0
                            else aTo[hp][:, csl])
                    nc.vector.tensor_tensor(dest, o_p[0:64, :], bc[:],
                                            op=ALU.mult)
            for hp in range(2):
                nc.gpsimd.dma_start(aT[hp][64:128, csl], aTo[hp][:, csl])

        def stage3(sc):
            scl = slice(sc * 512, (sc + 1) * 512)
            po = po_pool.tile([128, 8, 512], bf16, tag="po")
            for dd in range(8):
                pp = mm512.tile([128, 512], f32, tag="mm512")
                for kc2 in range(2):
                    nc.tensor.matmul(
                        pp[:],
                        wp_sb[:, kc2, dd * 128:(dd + 1) * 128],
                        aT[kc2][:, scl],
                        start=(kc2 == 0), stop=(kc2 == 1))
                nc.scalar.activation(po[:, dd, :], pp[:], AF.Identity,
                                     bias=bp_sb[:, dd:dd + 1])
            nc.sync.dma_start(
                outT_d.ap().rearrange("(a p) s -> p a s", a=8)[:, :, scl],
                po[:])

        # interleaved emission: keeps PE dense, lets exp start early;
        # proj chunks late so PE has fill work during the exp-bound tail
        stage1(0)
        stage1(1)
        stage2(0)
        stage1(2)
        stage2(1)
        stage1(3)
        stage2(2)
        stage3(0)
        stage3(1)
        stage2(3)
        stage3(2)
        stage3(3)

    nc.finalize()
    return nc


# pair-interleave: new row j within a head holds original dim PERM[j]
PERM = np.empty(64, np.int64)
PERM[0::2] = np.arange(32)
PERM[1::2] = np.arange(32) + 32


def make_core_inputs(inputs, core):
    """Host-side shard prep for one core."""
    import ml_dtypes
    bf = ml_dtypes.bfloat16
    b, g = core // 4, core % 4
    hidden = np.asarray(inputs["hidden_states"], dtype=np.float32)
    pos = np.asarray(inputs["position_ids"])
    caw = np.asarray(inputs["c_attn_w"], dtype=np.float32)
    cab = np.asarray(inputs["c_attn_b"], dtype=np.float32)
    cpw = np.asarray(inputs["c_proj_w"], dtype=np.float32)
    cpb = np.asarray(inputs["c_proj_b"], dtype=np.float32)

    cs = slice(g * HD, (g + 1) * HD)
    # per-head pair-interleaved column permutation for q and k
    hperm = np.concatenate([h * 64 + PERM for h in range(4)])
    wq = caw[:, cs][:, hperm]
    wk = caw[:, D + g * HD:D + (g + 1) * HD][:, hperm]
    wv = caw[:, 2 * D + g * HD:2 * D + (g + 1) * HD]
    wqkv = np.concatenate([wq, wk, wv], axis=1)

    bq = cab[cs][hperm]
    bk = cab[D + g * HD:D + (g + 1) * HD][hperm]
    bv = cab[2 * D + g * HD:2 * D + (g + 1) * HD]
    # bqk[:, jc]: jc0/1 = q head pairs, jc2/3 = k head pairs
    bqk = np.stack([bq[0:128], bq[128:256], bk[0:128], bk[128:256]],
                   axis=1).astype(np.float32)

    # rope tables in permuted transposed layout [128 rows = 2 heads x 64]
    inv_freq = (1.0 / (10000.0 **
                       (np.arange(0, 64, 2, dtype=np.float64) / 64.0)))
    theta = pos[b].astype(np.float64)[None, :] * inv_freq[:, None]  # [32,S]
    cosv = np.cos(theta)
    sinv = np.sin(theta)
    cos64 = np.empty((64, S), np.float64)
    sin64 = np.empty((64, S), np.float64)
    cos64[0::2] = cosv
    cos64[1::2] = cosv
    sin64[0::2] = -sinv      # row 2i   (orig dim i):    -sin
    sin64[1::2] = sinv       # row 2i+1 (orig dim i+32): +sin
    cosT = np.tile(cos64, (2, 1)).astype(bf)
    sinT = np.tile(sin64, (2, 1)).astype(bf)

    bp = (cpb if g == 0 else np.zeros_like(cpb)).reshape(8, 128).T.copy()

    r = np.arange(128)
    mask01 = (r[None, :] >= r[:, None]).astype(np.float32)
    mask2 = np.concatenate([mask01, mask01], axis=1)

    return {
        "hT": np.ascontiguousarray(hidden[b].T).astype(bf),
        "wqkv": np.ascontiguousarray(wqkv).astype(bf),
        "bqk": np.ascontiguousarray(bqk),
        "bv": bv[None, :].astype(bf),
        "cosT": cosT,
        "sinT": sinT,
        "wp": np.ascontiguousarray(cpw[cs, :]).astype(bf),
        "bp": np.ascontiguousarray(bp.astype(np.float32)),
        "mask2": mask2.astype(bf),
        "ones64": np.ones((128, 64), bf),
        "ones_row": np.ones((1, 128), bf),
    }


_NC_CACHE = {}


def run(inputs, trace=False, **spmd_kwargs):
    """Shard, execute on 8 cores, unshard. Returns (output, BassKernelResults)."""
    if "nc" not in _NC_CACHE:
        _NC_CACHE["nc"] = build_attention_nc(num_devices=8)
    nc = _NC_CACHE["nc"]
    in_maps = [make_core_inputs(inputs, c) for c in range(8)]
    res = run_bass_kernel_spmd(nc, in_maps, core_ids=list(range(8)),
                               trace=trace, **spmd_kwargs)
    outs = []
    for b in range(2):
        acc = np.zeros((D, S), np.float64)
        for g in range(4):
            acc += res.results[b * 4 + g]["outT"].astype(np.float64)
        outs.append(acc.T.astype(np.float32))
    return np.stack(outs, axis=0), res


def kernel(**inputs) -> np.ndarray:
    out, _ = run(inputs, trace=False)
    return out
